# revision 22
# baseline (speedup 1.0000x reference)
"""Trainium2 Bass kernel for nn_ConvLRUModel (2-block ConvLRU).

Sharding: 8 cores = (batch b in {0,1}) x (L-block j in {0..3}), T=16 timesteps
per core. All heavy compute is per-(b,l) except the LRU scan along L, which is
handled with local scans + one 4-core AllGather per block (carry exchange).

Math formulation (validated against the jax reference in numpy):
  The reference does fft2 over (H,W); everything between fft2/ifft2 commutes
  with the W-axis transform (channel mixes act on C; gamma/lambda depend on
  (C,Hf) only; the scan acts on L), so the W-FFT is dropped entirely:
    z = Wb @ fft_H(x)            (bias bb==0 in setup, asserted on host)
    z *= gamma[c,hf]
    y_t = lam[c,hf] * y_{t-1} + z_t        (w stays spatial)
    u = ifft_H(y);  v = Re(Wc @ u);  v = LN(v);  out = v + x ; FFN(out)
  last_hidden = fft_W(y_last) -- that tiny W-FFT runs on the host.

On-chip layouts (p = SBUF partition, f = free):
  X  [p=(t_lo,c)=128, f=(t_hi, h32, w32)]           t = 2*t_hi + t_lo
  A  [p=(w_lo4,hf32)=128, f=(t_hi8, w_grp8, t_lo2, o64)]   (scan state, r/i)
  FFN per-t padded tiles [p=channels, f=(H+2)*(W+2)=34*34]
Channel mixes contract c on partitions; the H-FFT contracts h on partitions via
block-diagonal DFT matmuls; PE transposes (128x128) move between the two.
Convs are 9 shifted accumulating matmuls; cin packs shift pairs (s, s+34) into
K=128 via a duplicated partition block holding x shifted by one spatial row.
All big matmuls run as float32r (full PE rate); transposes stay float32.
Carry correction y_t += lam^(t+1) * S is a sequential chain Q_t = lam*Q_{t-1}
interleaved with the per-t back end, so it pipelines into ifft/Cmix/FFN.
"""

import ml_dtypes
import numpy as np

import concourse.bacc as bacc
import concourse.bass as bass
import concourse.mybir as mybir
import concourse.tile as tile
from concourse.masks import make_identity

EMB, H, W = 64, 32, 32
B, L = 2, 64
FFN = 128
T = 16          # timesteps per core
TH, TL = 8, 2   # t = 2*t_hi + t_lo
NBLK = 2
EPS = 1e-5
NEG_SLOPE = 0.01
F32 = mybir.dt.float32
BF16 = mybir.dt.bfloat16
AF = mybir.ActivationFunctionType
OP = mybir.AluOpType

HP, WP = H + 2, W + 2          # padded spatial
PAD = HP * WP                  # 1156
INT0 = WP + 1                  # interior offset 35
SHIFTS = [-WP - 1, -WP, -WP + 1, -1, 0, 1, WP - 1, WP, WP + 1]


def _mkap(t, p0, pn, off, dims):
    full = t[:]
    pitch = full.ap[0][0]
    return bass.AP(full.tensor, full.offset + p0 * pitch + off,
                   [[pitch, pn]] + [list(d) for d in dims])


_BF_KEYS = {'fmr', 'fmi', 'fmin', 'gmr', 'gmi', 'gmin', 'wbr', 'wbi',
            'wcr', 'wcin', 'cinp', 'cins', 'hidw', 'coutw'}


def _cdt(key):
    return BF16 if key.rstrip('01') in _BF_KEYS else F32


def _bf(a):
    return np.asarray(a).astype(ml_dtypes.bfloat16)


# ---------------------------------------------------------------------------
# host-side constant packing
# ---------------------------------------------------------------------------

def _lam_gam(blk):
    nu = np.exp(np.asarray(blk['nu_log']))
    th = np.exp(np.asarray(blk['theta_log']))
    gam = np.exp(np.asarray(blk['gamma_log'])).astype(np.float32)
    lam = np.exp(-nu + 1j * th).astype(np.complex64)
    return lam, gam


def _field(v):  # v [C,Hf] -> [p=(w_lo,hf)=128, f=o64] (w-independent, compact)
    arr = np.zeros((4, 32, 64), np.float32)
    arr[:, :, :] = v.T[None, :, :]
    return arr.reshape(128, 64)


def _pack_consts(params):
    c = {}
    FH = np.fft.fft(np.eye(H)).astype(np.complex64)
    GH = (np.conj(FH).T / H).astype(np.complex64)

    def blkdiag4(M):  # lhsT[k=(w_lo,h), m=(w_lo,hf)] = M[hf,h]
        out = np.zeros((128, 128), np.float32)
        for wl in range(4):
            out[wl * 32:(wl + 1) * 32, wl * 32:(wl + 1) * 32] = M.T
        return out

    c['fmr'] = blkdiag4(FH.real)
    c['fmi'] = blkdiag4(FH.imag)
    c['fmin'] = -c['fmi']
    c['gmr'] = blkdiag4(GH.real)
    c['gmi'] = blkdiag4(GH.imag)
    c['gmin'] = -c['gmi']
    c['ones'] = np.ones((128, 128), np.float32)

    for bi, blk in enumerate(params['blocks']):
        lam, gam = _lam_gam(blk)
        Bw_r = np.asarray(blk['Bw_r'], np.float32)
        Bw_i = np.asarray(blk['Bw_i'], np.float32)
        Cw_r = np.asarray(blk['Cw_r'], np.float32)
        Cw_i = np.asarray(blk['Cw_i'], np.float32)

        wb_r = np.zeros((128, 128), np.float32)
        wb_i = np.zeros((128, 128), np.float32)
        for tl in range(2):
            s = slice(tl * 64, tl * 64 + 64)
            wb_r[s, s] = Bw_r.T     # lhsT[k=c, m=o] = Wb[o,c]
            wb_i[s, s] = Bw_i.T
        c[f'wbr{bi}'] = wb_r
        c[f'wbi{bi}'] = wb_i
        c[f'wcr{bi}'] = np.vstack([Cw_r.T, Cw_r.T]).copy()
        c[f'wcin{bi}'] = np.vstack([-Cw_i.T, -Cw_i.T]).copy()
        c[f'lamr{bi}'] = _field(lam.real)
        c[f'lami{bi}'] = _field(lam.imag)
        c[f'gam{bi}'] = _field(gam)

        cin_w = np.asarray(blk['cin_w'], np.float32)
        hid_w = np.asarray(blk['hid'][0]['w'], np.float32)
        cout_w = np.asarray(blk['cout_w'], np.float32)

        def tap(wt, s):  # lhsT [Cin, Cout] for shift s
            k = SHIFTS.index(s)
            dy, dx = k // 3 - 1, k % 3 - 1
            return wt[:, :, dy + 1, dx + 1].T.copy()

        cinp = np.zeros((128, 3 * 128), np.float32)
        for k, s in enumerate((-WP - 1, -WP, -WP + 1)):
            cinp[:64, k * 128:(k + 1) * 128] = tap(cin_w, s)
            cinp[64:, k * 128:(k + 1) * 128] = tap(cin_w, s + WP)
        c[f'cinp{bi}'] = cinp
        cins = np.zeros((64, 3 * 128), np.float32)
        for k, s in enumerate((WP - 1, WP, WP + 1)):
            cins[:, k * 128:(k + 1) * 128] = tap(cin_w, s)
        c[f'cins{bi}'] = cins
        hw = np.zeros((128, 9 * 128), np.float32)
        for k, s in enumerate(SHIFTS):
            hw[:, k * 128:(k + 1) * 128] = tap(hid_w, s)
        c[f'hidw{bi}'] = hw
        cw = np.zeros((128, 9 * 64), np.float32)
        for k, s in enumerate(SHIFTS):
            cw[:, k * 64:(k + 1) * 64] = tap(cout_w, s)
        c[f'coutw{bi}'] = cw
    for k in list(c):
        if k.rstrip('01') in _BF_KEYS:
            c[k] = _bf(c[k])
    return c


def _pack_core_consts(params, j):
    out = {}
    for bi, blk in enumerate(params['blocks']):
        lam, _ = _lam_gam(blk)
        lam16 = lam ** T
        cwr = np.zeros((4, 32, 4, 64), np.float32)  # [wl, hf, i, o]
        cwi = np.zeros((4, 32, 4, 64), np.float32)
        for i in range(j):
            w = lam16 ** (j - 1 - i)
            cwr[:, :, i, :] = w.real.T[None]
            cwi[:, :, i, :] = w.imag.T[None]
        out[f'cwr{bi}'] = cwr.reshape(128, 256)
        out[f'cwi{bi}'] = cwi.reshape(128, 256)
    return out


_SHARED_SHAPES = {
    'fmr': (128, 128), 'fmi': (128, 128), 'fmin': (128, 128),
    'gmr': (128, 128), 'gmi': (128, 128), 'gmin': (128, 128),
    'ones': (128, 128),
}
_BLK_SHAPES = {
    'wbr': (128, 128), 'wbi': (128, 128),
    'wcr': (128, 64), 'wcin': (128, 64),
    'lamr': (128, 64), 'lami': (128, 64), 'gam': (128, 64),
    'cwr': (128, 256), 'cwi': (128, 256),
    'cinp': (128, 384), 'cins': (64, 384),
    'hidw': (128, 1152), 'coutw': (128, 576),
}


# ---------------------------------------------------------------------------
# the Tile program
# ---------------------------------------------------------------------------

def _emit(nc):
    xdram = nc.dram_tensor('x', [128, 8192], F32, kind='ExternalInput')
    outdram = nc.dram_tensor('out', [T, EMB * H * W], F32, kind='ExternalOutput')
    lhdram = [nc.dram_tensor(f'lh{bi}', [128, 1024], F32, kind='ExternalOutput')
              for bi in range(NBLK)]
    cdram = {}
    for k, v in _SHARED_SHAPES.items():
        cdram[k] = nc.dram_tensor(k, list(v), _cdt(k), kind='ExternalInput')
    for bi in range(NBLK):
        for k, v in _BLK_SHAPES.items():
            cdram[f'{k}{bi}'] = nc.dram_tensor(f'{k}{bi}', list(v),
                                               _cdt(k), kind='ExternalInput')

    from contextlib import ExitStack
    with tile.TileContext(nc) as tc, ExitStack() as es:
        cpool = es.enter_context(tc.tile_pool(name='consts', bufs=1))
        bpool = es.enter_context(tc.tile_pool(name='bconsts', bufs=1))
        dpool = es.enter_context(tc.tile_pool(name='data', bufs=1))
        xqpool = es.enter_context(tc.tile_pool(name='xq', bufs=2))
        zpool = es.enter_context(tc.tile_pool(name='zchunk', bufs=1))
        upool = es.enter_context(tc.tile_pool(name='upair', bufs=1))
        vpool = es.enter_context(tc.tile_pool(name='vtile', bufs=2))
        v1pool = es.enter_context(tc.tile_pool(name='v1tile', bufs=1))
        fpool = es.enter_context(tc.tile_pool(name='ffn', bufs=2))
        f1pool = es.enter_context(tc.tile_pool(name='ffn1', bufs=1))
        tpool = es.enter_context(tc.tile_pool(name='tmp', bufs=2))
        qpool = es.enter_context(tc.tile_pool(name='qchain', bufs=2))
        xrpool = es.enter_context(tc.tile_pool(name='xres', bufs=2))
        fgpool = es.enter_context(tc.tile_pool(name='fg', bufs=1))
        abpool = es.enter_context(tc.tile_pool(name='ab', bufs=2))
        spool = es.enter_context(tc.tile_pool(name='stats', bufs=2))
        wpool = es.enter_context(tc.tile_pool(name='lnw', bufs=4))
        pbig = es.enter_context(tc.tile_pool(name='psbig', bufs=2, space='PSUM'))
        psml = es.enter_context(tc.tile_pool(name='pssml', bufs=4, space='PSUM'))
        drampool = es.enter_context(tc.tile_pool(name='dramcc', bufs=1, space='DRAM'))

        CS = {}
        for k, shp in _SHARED_SHAPES.items():
            ct = cpool.tile(list(shp), _cdt(k), tag=k)
            nc.gpsimd.dma_start(ct[:], cdram[k][:])
            CS[k] = ct
        ident = cpool.tile([128, 128], BF16, tag='ident')
        make_identity(nc, ident[:])
        CB = []
        for bi in range(NBLK):
            cb = {}
            for k, shp in _BLK_SHAPES.items():
                ct = bpool.tile(list(shp), _cdt(k), tag=f'{k}{bi}')
                nc.gpsimd.dma_start(ct[:], cdram[f'{k}{bi}'][:])
                cb[k] = ct
            CB.append(cb)

        Ar = dpool.tile([128, 8192], F32, tag='Ar')
        Ai = dpool.tile([128, 8192], F32, tag='Ai')
        xmid = [drampool.tile([64, 1024], F32, tag=f'xmid{t}', name=f'xmid{t}')
                for t in range(T)]

        def t_slice(tl_, th_, tt):
            return _mkap(tt, 0, 128, th_ * 1024 + tl_ * 64, [[128, 8], [1, 64]])

        def bc_o(tt, off):     # [p, o64 at off] -> broadcast [p,(wg8,o64)]
            return _mkap(tt, 0, 128, off, [[0, 8], [1, 64]])

        def v2(tt):            # [128,512] tile -> [p,(wg8,o64)] view
            return _mkap(tt, 0, 128, 0, [[64, 8], [1, 64]])

        for bi in range(NBLK):
            C = dict(CS)
            C.update(CB[bi])

            # ---------------- front: Bmix -> T1 -> fft -> A ----------------
            for th_ in range(TH):
                XQ = xqpool.tile([128, 1024], F32, tag='XQ')
                if bi == 0:
                    nc.gpsimd.dma_start(XQ[:], xdram[:, th_ * 1024:(th_ + 1) * 1024])
                else:
                    nc.gpsimd.dma_start(_mkap(XQ, 0, 64, 0, [[1, 1024]]),
                                      xmid[2 * th_][:])
                    nc.gpsimd.dma_start(_mkap(XQ, 64, 64, 0, [[1, 1024]]),
                                      xmid[2 * th_ + 1][:])
                XQb = xqpool.tile([128, 1024], BF16, tag='XQb')
                nc.scalar.activation(XQb[:], XQ[:], AF.Copy)
                if True:
                    for wh in range(2):
                        zf = {}
                        for part, wname in (('r', 'wbr'), ('i', 'wbi')):
                            psB = psml.tile([128, 512], F32, tag='ps_s')
                            rhs = _mkap(XQb, 0, 128, wh * 16,
                                        [[1, 16], [32, 32]])
                            nc.tensor.matmul(psB[:], C[wname][:], rhs,
                                             start=True, stop=True)
                            zc = zpool.tile([128, 512], BF16, tag='z' + part)
                            nc.vector.tensor_copy(zc[:], psB[:])
                            psT = psml.tile([128, 512], BF16, tag='ps_s')
                            for k in range(4):
                                nc.tensor.transpose(
                                    psT[:, k * 128:(k + 1) * 128],
                                    zc[:, k * 128:(k + 1) * 128], ident[:])
                            zfc = zpool.tile([128, 512], BF16, tag='zf' + part)
                            nc.vector.tensor_copy(zfc[:], psT[:])
                            zf[part] = zfc
                        gview = _mkap(C['gam'], 0, 128, 0,
                                      [[0, 4], [0, 2], [1, 64]])
                        for dst, l1, l2 in ((Ar, 'fmr', 'fmin'),
                                            (Ai, 'fmi', 'fmr')):
                            psF = psml.tile([128, 512], F32, tag='ps_s')
                            nc.tensor.matmul(psF[:], C[l1][:],
                                             zf['r'][:], start=True, stop=False)
                            nc.tensor.matmul(psF[:], C[l2][:],
                                             zf['i'][:], start=False, stop=True)
                            pv = _mkap(psF, 0, 128, 0,
                                       [[128, 4], [64, 2], [1, 64]])
                            dv = _mkap(dst, 0, 128, th_ * 1024 + wh * 512,
                                       [[128, 4], [64, 2], [1, 64]])
                            nc.vector.tensor_mul(dv, pv, gview)

            # ---------------- local scan (t = 1..15) ----------------
            lr = bc_o(C['lamr'], 0)
            li = bc_o(C['lami'], 0)
            for t in range(1, T):
                tl_, th_ = t % 2, t // 2
                pl_, ph_ = (t - 1) % 2, (t - 1) // 2
                yr_p = t_slice(pl_, ph_, Ar)
                yi_p = t_slice(pl_, ph_, Ai)
                t1 = tpool.tile([128, 512], F32, tag='sc1')
                t2 = tpool.tile([128, 512], F32, tag='sc2')
                nc.vector.tensor_mul(v2(t1), lr, yr_p)
                nc.gpsimd.tensor_mul(v2(t2), li, yi_p)
                nc.vector.tensor_sub(v2(t1), v2(t1), v2(t2))
                nc.vector.tensor_add(t_slice(tl_, th_, Ar),
                                     t_slice(tl_, th_, Ar), v2(t1))
                t3 = tpool.tile([128, 512], F32, tag='sc1')
                t4 = tpool.tile([128, 512], F32, tag='sc2')
                nc.vector.tensor_mul(v2(t3), lr, yi_p)
                nc.gpsimd.tensor_mul(v2(t4), li, yr_p)
                nc.vector.tensor_add(v2(t3), v2(t3), v2(t4))
                nc.vector.tensor_add(t_slice(tl_, th_, Ai),
                                     t_slice(tl_, th_, Ai), v2(t3))

            # ---------------- carry exchange ----------------
            ccin = drampool.tile([128, 1024], F32, tag=f'ccin{bi}')
            ccout = drampool.tile([512, 1024], F32, tag=f'ccout{bi}')
            nc.gpsimd.dma_start(_mkap(ccin, 0, 128, 0, [[1, 512]]),
                              t_slice(1, 7, Ar))
            nc.gpsimd.dma_start(_mkap(ccin, 0, 128, 512, [[1, 512]]),
                              t_slice(1, 7, Ai))
            nc.gpsimd.collective_compute(
                'AllGather', OP.bypass,
                replica_groups=[[0, 1, 2, 3], [4, 5, 6, 7]],
                ins=[ccin[:].opt()], outs=[ccout[:].opt()])
            Sr = tpool.tile([128, 512], F32, tag='Sr')
            Si = tpool.tile([128, 512], F32, tag='Si')
            nc.gpsimd.memset(Sr[:], 0.0)
            nc.gpsimd.memset(Si[:], 0.0)
            for i in range(3):   # S_j only ever needs F_0..F_2
                Fgi = fgpool.tile([128, 1024], F32, tag='Fg')
                nc.gpsimd.dma_start(Fgi[:], ccout[i * 128:(i + 1) * 128, :])
                cr = bc_o(C['cwr'], i * 64)
                ci = bc_o(C['cwi'], i * 64)
                fr = _mkap(Fgi, 0, 128, 0, [[64, 8], [1, 64]])
                fi = _mkap(Fgi, 0, 128, 512, [[64, 8], [1, 64]])
                ta = tpool.tile([128, 512], F32, tag='sc1')
                tb = tpool.tile([128, 512], F32, tag='sc2')
                nc.gpsimd.tensor_mul(v2(ta), cr, fr)
                nc.gpsimd.tensor_mul(v2(tb), ci, fi)
                nc.gpsimd.tensor_sub(v2(ta), v2(ta), v2(tb))
                nc.gpsimd.tensor_add(v2(Sr), v2(Sr), v2(ta))
                nc.vector.tensor_mul(v2(ta), cr, fi)
                nc.vector.tensor_mul(v2(tb), ci, fr)
                nc.vector.tensor_add(v2(ta), v2(ta), v2(tb))
                nc.vector.tensor_add(v2(Si), v2(Si), v2(ta))

            # ------------- back end, with correction chain interleaved -----
            Qp = (Sr, Si)
            for tp in range(TH):
                for tl_ in range(2):
                    t = 2 * tp + tl_
                    # Q_t = lam * Q_{t-1};  y_t += Q_t
                    Qr = qpool.tile([128, 512], F32, tag='Qr')
                    Qi = qpool.tile([128, 512], F32, tag='Qi')
                    ta = tpool.tile([128, 512], F32, tag='sc1')
                    tb = tpool.tile([128, 512], F32, tag='sc2')
                    nc.gpsimd.tensor_mul(v2(ta), lr, v2(Qp[0]))
                    nc.gpsimd.tensor_mul(v2(tb), li, v2(Qp[1]))
                    nc.gpsimd.tensor_sub(v2(Qr), v2(ta), v2(tb))
                    nc.vector.tensor_mul(v2(ta), lr, v2(Qp[1]))
                    nc.vector.tensor_mul(v2(tb), li, v2(Qp[0]))
                    nc.vector.tensor_add(v2(Qi), v2(ta), v2(tb))
                    nc.vector.tensor_add(t_slice(tl_, tp, Ar),
                                         t_slice(tl_, tp, Ar), v2(Qr))
                    nc.vector.tensor_add(t_slice(tl_, tp, Ai),
                                         t_slice(tl_, tp, Ai), v2(Qi))
                    Qp = (Qr, Qi)

                Ur = upool.tile([128, 1024], BF16, tag='Ur')
                Ui = upool.tile([128, 1024], BF16, tag='Ui')
                for tl_ in range(2):
                    Abr = abpool.tile([128, 512], BF16, tag='Abr')
                    Abi = abpool.tile([128, 512], BF16, tag='Abi')
                    abv = lambda tt: _mkap(tt, 0, 128, 0, [[64, 8], [1, 64]])
                    nc.scalar.activation(abv(Abr), t_slice(tl_, tp, Ar), AF.Copy)
                    nc.scalar.activation(abv(Abi), t_slice(tl_, tp, Ai), AF.Copy)
                    for dst, l1, l2 in ((Ur, 'gmr', 'gmin'), (Ui, 'gmi', 'gmr')):
                        psU = psml.tile([128, 512], F32, tag='ps_s')
                        nc.tensor.matmul(psU[:], C[l1][:], Abr[:],
                                         start=True, stop=False)
                        nc.tensor.matmul(psU[:], C[l2][:], Abi[:],
                                         start=False, stop=True)
                        dv = _mkap(dst, 0, 128, tl_ * 64, [[128, 8], [1, 64]])
                        nc.vector.tensor_copy(dv, _mkap(psU, 0, 128, 0,
                                                        [[64, 8], [1, 64]]))
                V2r = v1pool.tile([128, 1024], BF16, tag='V2r')
                V2i = v1pool.tile([128, 1024], BF16, tag='V2i')
                for src, dstt in ((Ur, V2r), (Ui, V2i)):
                    for half in range(2):
                        psT = psml.tile([128, 512], BF16, tag='ps_s')
                        for k in range(4):
                            wg = half * 4 + k
                            nc.tensor.transpose(
                                psT[:, k * 128:(k + 1) * 128],
                                src[:, wg * 128:(wg + 1) * 128], ident[:])
                        pv = _mkap(psT, 0, 128, 0, [[128, 4], [32, 4], [1, 32]])
                        dv = _mkap(dstt, 0, 128, half * 16,
                                   [[4, 4], [1, 4], [32, 32]])
                        nc.vector.tensor_copy(dv, pv)
                for tl_ in range(2):
                    t = 2 * tp + tl_
                    _ffn_t(nc, C, CS, bi, t, tl_, tp, V2r, V2i,
                           xdram if bi == 0 else None, xmid,
                           vpool, v1pool, fpool, f1pool, xrpool, spool, wpool,
                           pbig, psml, outdram)

            # last_hidden export (hf-domain; host applies the W-FFT)
            nc.gpsimd.dma_start(_mkap(lhdram[bi], 0, 128, 0, [[1, 512]]),
                              t_slice(1, 7, Ar))
            nc.gpsimd.dma_start(_mkap(lhdram[bi], 0, 128, 512, [[1, 512]]),
                              t_slice(1, 7, Ai))
    return nc


def _ffn_t(nc, C, CS, bi, t, tl_, tp, V2r, V2i, xdram, xmid,
           vpool, v1pool, fpool, f1pool, xrpool, spool, wpool, pbig, psml,
           outdram):
    th_ = tp
    # residual x slice for this t, re-read from DRAM
    xres = xrpool.tile([64, 1024], F32, tag='xres')
    if xdram is not None:
        nc.gpsimd.dma_start(xres[:],
                          bass.AP(xdram[:].tensor, tl_ * 64 * 8192 + th_ * 1024,
                                  [[8192, 64], [1, 1024]]))
    else:
        nc.gpsimd.dma_start(xres[:], xmid[t][:])

    # ---- Cmix: v = Wcr @ ur - Wci @ ui  (K=64 partition half) ----
    psV = pbig.tile([64, 1024], F32, tag='ps_b')
    for ch in range(2):
        o = ch * 512
        rr = _mkap(V2r, tl_ * 64, 64, o, [[1, 512]])
        ri = _mkap(V2i, tl_ * 64, 64, o, [[1, 512]])
        wsl = slice(tl_ * 64, tl_ * 64 + 64)
        nc.tensor.matmul(psV[:, o:o + 512], C['wcr'][wsl, :], rr,
                         start=True, stop=False)
        nc.tensor.matmul(psV[:, o:o + 512], C['wcin'][wsl, :], ri,
                         start=False, stop=True)
    # ---- LN1 on v ; out1 = LN(v) + x ----
    st = spool.tile([128, 8], F32, tag='st')
    v = v1pool.tile([64, 1024], F32, tag='vt64')
    nc.scalar.activation(v[:], psV[:], AF.Copy, accum_out=st[0:64, 0:1])
    sq1 = pbig.tile([64, 1024], F32, tag='ps_b')
    nc.scalar.activation(sq1[:], v[:], AF.Square, accum_out=st[0:64, 1:2])
    inv1, nb1 = _ln_stats(nc, CS, st, 0, 64, float(EMB * H * W), psml, wpool)
    out1f = vpool.tile([64, 1024], F32, tag='out1f')
    nc.scalar.activation(out1f[:], v[:], AF.Identity, bias=nb1, scale=inv1)
    nc.vector.tensor_add(out1f[:], out1f[:], xres[:])
    out1 = fpool.tile([128, PAD], BF16, tag='out1')
    nc.gpsimd.memset(out1[:], 0.0)
    o1i = _mkap(out1, 0, 64, INT0, [[WP, 32], [1, 32]])
    nc.scalar.activation(o1i, _mkap(out1f, 0, 64, 0, [[32, 32], [1, 32]]),
                         AF.Copy)
    # dup copy for cin shift-pairing: p 64..127 = out1 shifted +WP
    nc.gpsimd.dma_start(_mkap(out1, 64, 64, 0, [[1, PAD - WP]]),
                      _mkap(out1, 0, 64, WP, [[1, PAD - WP]]))
    nc.gpsimd.memset(_mkap(out1, 64, 64, PAD - WP, [[1, WP]]), 0.0)

    # ---- cin (64->128): 3 paired + 3 single shifts ----
    psC = pbig.tile([128, 1024], F32, tag='ps_b')
    for ch in range(2):
        co = ch * 512
        ro = INT0 + ch * 16 * WP
        for k, s in enumerate((-WP - 1, -WP, -WP + 1)):
            nc.tensor.matmul(psC[:, co:co + 512],
                             C['cinp'][:, k * 128:(k + 1) * 128],
                             _mkap(out1, 0, 128, ro + s, [[WP, 16], [1, 32]]),
                             start=(k == 0), stop=False)
        for k, s in enumerate((WP - 1, WP, WP + 1)):
            nc.tensor.matmul(psC[:, co:co + 512],
                             C['cins'][:, k * 128:(k + 1) * 128],
                             _mkap(out1, 0, 64, ro + s, [[WP, 16], [1, 32]]),
                             start=False, stop=(k == 2))
    yt = fpool.tile([128, PAD], BF16, tag='yt')
    nc.gpsimd.memset(yt[:], 0.0)
    for ch in range(2):
        nc.scalar.activation(
            _mkap(yt, 0, 128, INT0 + ch * 16 * WP, [[WP, 16], [1, 32]]),
            psC[:, ch * 512:(ch + 1) * 512], AF.Lrelu, alpha=NEG_SLOPE)

    # ---- hid (128->128) + LN + residual ----
    psH = pbig.tile([128, 1024], F32, tag='ps_b')
    for ch in range(2):
        co = ch * 512
        ro = INT0 + ch * 16 * WP
        for k, s in enumerate(SHIFTS):
            nc.tensor.matmul(psH[:, co:co + 512],
                             C['hidw'][:, k * 128:(k + 1) * 128],
                             _mkap(yt, 0, 128, ro + s, [[WP, 16], [1, 32]]),
                             start=(k == 0), stop=(k == 8))
    y2 = v1pool.tile([128, 1024], BF16, tag='y2')
    nc.scalar.activation(y2[:], psH[:], AF.Lrelu, alpha=NEG_SLOPE,
                         accum_out=st[:, 2:3])
    sq2 = pbig.tile([128, 1024], F32, tag='ps_b')
    nc.scalar.activation(sq2[:], y2[:], AF.Square, accum_out=st[:, 3:4])
    inv2, nb2 = _ln_stats(nc, CS, st, 2, 128, float(FFN * H * W), psml, wpool)
    y2n = v1pool.tile([128, 1024], BF16, tag='scr')
    nc.scalar.activation(y2n[:], y2[:], AF.Identity, bias=nb2, scale=inv2)
    yti = _mkap(yt, 0, 128, INT0, [[WP, 32], [1, 32]])
    nc.vector.tensor_add(yti, yti, _mkap(y2n, 0, 128, 0, [[32, 32], [1, 32]]))

    # ---- cout (128->64) + LN2 + residual + store ----
    psO = pbig.tile([64, 1024], F32, tag='ps_b')
    for ch in range(2):
        co = ch * 512
        ro = INT0 + ch * 16 * WP
        for k, s in enumerate(SHIFTS):
            nc.tensor.matmul(psO[:, co:co + 512],
                             C['coutw'][:, k * 64:(k + 1) * 64],
                             _mkap(yt, 0, 128, ro + s, [[WP, 16], [1, 32]]),
                             start=(k == 0), stop=(k == 8))
    v3 = v1pool.tile([64, 1024], F32, tag='vt64')
    nc.scalar.activation(v3[:], psO[:], AF.Copy, accum_out=st[0:64, 4:5])
    sq3 = pbig.tile([64, 1024], F32, tag='ps_b')
    nc.scalar.activation(sq3[:], v3[:], AF.Square, accum_out=st[0:64, 5:6])
    inv3, nb3 = _ln_stats(nc, CS, st, 4, 64, float(EMB * H * W), psml, wpool)
    xout = vpool.tile([64, 1024], F32, tag='xout')
    nc.scalar.activation(xout[:], v3[:], AF.Identity, bias=nb3, scale=inv3)
    nc.vector.tensor_add(xout[:], xout[:], out1f[:])
    if bi == NBLK - 1:
        nc.gpsimd.dma_start(
            bass.AP(outdram[:].tensor, t * EMB * H * W, [[1024, 64], [1, 1024]]),
            xout[:])
    else:
        nc.gpsimd.dma_start(xmid[t][:], xout[:])


def _ln_stats(nc, CS, st, col, p, n, psml, wpool):
    ones = CS['ones']
    psS = psml.tile([128, 8], F32, tag='ps_s')
    nc.tensor.matmul(psS[0:p, 0:2], ones[0:p, 0:p], st[0:p, col:col + 2],
                     start=True, stop=True)
    w = wpool.tile([128, 8], F32, tag='lnw')
    nc.vector.tensor_scalar(w[0:p, 0:1], psS[0:p, 0:1], -1.0 / n, None, OP.mult)
    nc.vector.tensor_scalar(w[0:p, 1:2], psS[0:p, 1:2], 1.0 / n, None, OP.mult)
    nc.vector.tensor_mul(w[0:p, 2:3], w[0:p, 0:1], w[0:p, 0:1])
    nc.vector.tensor_sub(w[0:p, 1:2], w[0:p, 1:2], w[0:p, 2:3])
    nc.vector.tensor_scalar(w[0:p, 1:2], w[0:p, 1:2], EPS, None, OP.add)
    nc.vector.reciprocal(w[0:p, 1:2], w[0:p, 1:2])
    nc.scalar.activation(w[0:p, 3:4], w[0:p, 1:2], AF.Sqrt)
    nc.vector.tensor_mul(w[0:p, 4:5], w[0:p, 0:1], w[0:p, 3:4])
    return w[0:p, 3:4], w[0:p, 4:5]


# ---------------------------------------------------------------------------
# host driver
# ---------------------------------------------------------------------------

def _fast_path_ok(params):
    try:
        for blk in params['blocks']:
            for k in ('Bb_r', 'Bb_i', 'Cb_r', 'Cb_i', 'cin_b', 'cout_b'):
                if np.abs(np.asarray(blk[k])).max() > 0:
                    return False
            for k in ('ln1_w', 'ln2_w'):
                if np.abs(np.asarray(blk[k]) - 1).max() > 0:
                    return False
            for k in ('ln1_b', 'ln2_b'):
                if np.abs(np.asarray(blk[k])).max() > 0:
                    return False
            if len(blk['hid']) != 1:
                return False
            hp = blk['hid'][0]
            if (np.abs(np.asarray(hp['b'])).max() > 0
                    or np.abs(np.asarray(hp['ln_w']) - 1).max() > 0
                    or np.abs(np.asarray(hp['ln_b'])).max() > 0):
                return False
        return True
    except Exception:
        return False


def _numpy_fallback(x, params):
    """Exact (slow) numpy replica of the reference; used only if the harness
    supplies params outside the structure produced by setup_inputs()."""
    def ln(v, w, b):
        m = v.mean(axis=(-3, -2, -1), keepdims=True)
        var = v.var(axis=(-3, -2, -1), keepdims=True)
        return (v - m) / np.sqrt(var + EPS) * w + b

    def conv3(v, w, b):
        N, Ci, Hh, Ww = v.shape
        xp = np.zeros((N, Ci, Hh + 2, Ww + 2), v.dtype)
        xp[:, :, 1:-1, 1:-1] = v
        out = np.zeros((N, w.shape[0], Hh, Ww), np.float32)
        for dy in (-1, 0, 1):
            for dx in (-1, 0, 1):
                out += np.einsum('oc,nchw->nohw', w[:, :, dy + 1, dx + 1],
                                 xp[:, :, 1 + dy:1 + dy + Hh,
                                    1 + dx:1 + dx + Ww])
        return out + b[None, :, None, None]

    x = np.asarray(x, np.float32)
    lhs = []
    for blk in params['blocks']:
        lam, gam = _lam_gam(blk)
        Wb = np.asarray(blk['Bw_r']) + 1j * np.asarray(blk['Bw_i'])
        Wc = np.asarray(blk['Cw_r']) + 1j * np.asarray(blk['Cw_i'])
        bb = np.asarray(blk['Bb_r']) + 1j * np.asarray(blk['Bb_i'])
        cb = np.asarray(blk['Cb_r']) + 1j * np.asarray(blk['Cb_i'])
        h = np.fft.fft2(x.astype(np.complex64))
        h = np.einsum('blchw,oc->blohw', h, Wb) + bb[None, None, :, None, None]
        h = h * gam[None, None, :, :, None]
        y = np.empty_like(h)
        s = np.zeros(h[:, 0].shape, np.complex64)
        for t in range(h.shape[1]):
            s = lam[None, :, :, None] * s + h[:, t]
            y[:, t] = s
        lhs.append(y[:, -1:].copy())
        u = np.fft.ifft2(y)
        vv = (np.einsum('blchw,oc->blohw', u, Wc)
              + cb[None, None, :, None, None]).real.astype(np.float32)
        vv = ln(vv, np.asarray(blk['ln1_w']), np.asarray(blk['ln1_b']))
        x1 = vv + x
        bl = x1.shape[0] * x1.shape[1]
        yy = conv3(x1.reshape(bl, EMB, H, W), np.asarray(blk['cin_w']),
                   np.asarray(blk['cin_b']))
        yy = np.where(yy >= 0, yy, NEG_SLOPE * yy).reshape(
            x1.shape[0], x1.shape[1], FFN, H, W)
        for hp in blk['hid']:
            y2 = conv3(yy.reshape(bl, FFN, H, W), np.asarray(hp['w']),
                       np.asarray(hp['b'])).reshape(yy.shape)
            y2 = np.where(y2 >= 0, y2, NEG_SLOPE * y2)
            y2 = ln(y2, np.asarray(hp['ln_w']), np.asarray(hp['ln_b']))
            yy = y2 + yy
        y3 = conv3(yy.reshape(bl, FFN, H, W), np.asarray(blk['cout_w']),
                   np.asarray(blk['cout_b'])).reshape(x1.shape)
        y3 = ln(y3, np.asarray(blk['ln2_w']), np.asarray(blk['ln2_b']))
        x = (y3 + x1).astype(np.float32)
    return x, tuple(lhs)


_CACHE = {}


def kernel(x, params):
    x = np.asarray(x, np.float32)
    if not _fast_path_ok(params):
        return _numpy_fallback(x, params)

    from concourse.bass_utils import run_bass_kernel_spmd

    consts = _pack_consts(params)
    in_maps = []
    for core in range(8):
        b, j = divmod(core, 4)
        xc = np.asarray(x[b, j * T:(j + 1) * T], np.float32)     # [T,C,H,W]
        xp = xc.reshape(TH, TL, EMB, H * W).transpose(1, 2, 0, 3)
        m = {'x': np.ascontiguousarray(xp.reshape(128, 8192))}
        m.update(consts)
        m.update(_pack_core_consts(params, j))
        in_maps.append(m)

    if 'nc' not in _CACHE:
        nc = bacc.Bacc(None, target_bir_lowering=False, num_devices=8)
        _emit(nc)
        nc.finalize()
        _CACHE['nc'] = nc
    res = run_bass_kernel_spmd(_CACHE['nc'], in_maps,
                               core_ids=list(range(8))).results

    out = np.zeros((B, L, EMB, H, W), np.float32)
    for core in range(8):
        b, j = divmod(core, 4)
        out[b, j * T:(j + 1) * T] = res[core]['out'].reshape(T, EMB, H, W)
    FW = np.fft.fft(np.eye(W)).astype(np.complex64)
    lhs = []
    for bi in range(NBLK):
        lh = np.zeros((B, 1, EMB, H, W), np.complex64)
        for b in range(B):
            raw = res[b * 4 + 3][f'lh{bi}']                   # [128, 1024]
            yr = raw[:, 0:512].reshape(4, 32, 8, 64)          # [wl,hf,wg,o]
            yi = raw[:, 512:1024].reshape(4, 32, 8, 64)
            y = (yr + 1j * yi).transpose(3, 1, 2, 0)          # [o,hf,wg,wl]
            y = np.ascontiguousarray(y).reshape(EMB, H, W)    # w = wg*4+wl
            lh[b, 0] = np.einsum('gw,cfw->cfg', FW, y)
        lhs.append(lh)
    return out, tuple(lhs)
